# revision 12
# baseline (speedup 1.0000x reference)
import sys, types
sys.path.insert(0, "/opt/trn_rl_repo")
import numpy as np
import ml_dtypes

BF16 = ml_dtypes.bfloat16


def _install_ntff_shim():
    try:
        import antenv  # noqa
        from trn_agent_boot.trn_boot import _ntff_profile_via_ctypes
        hook = _ntff_profile_via_ctypes('/opt/axon/libaxon_pjrt.so')
        m = types.ModuleType("antenv.axon_hooks")
        m.get_axon_ntff_profile_hook = lambda: hook
        m.set_axon_ntff_profile_hook = lambda h: None
        sys.modules["antenv.axon_hooks"] = m
    except Exception:
        pass
_install_ntff_shim()

from concourse import bass, mybir, tile, bacc
from concourse.bass_utils import run_bass_kernel_spmd

FP = mybir.dt.float32
BF = mybir.dt.bfloat16
I16 = mybir.dt.int16

N, IN, H1, C1, OUT = 50000, 256, 4, 32, 40
NC_ = 8
NPC = N // NC_              # 6250 dsts per core
NG = 49                     # groups of 128 dsts per core
SPLIT = 24960               # table A = nodes [0, SPLIT), B = [SPLIT, N)
NTILE = 391                 # ceil(N/128)
NPAD = NTILE * 128          # 50048
AROWS = SPLIT               # A table real rows; dummy at AROWS
BROWS = NPAD - SPLIT        # 25088 B rows (incl 48 fake); dummy at BROWS
ELEM1, ELEM2 = 256, 128     # gather elem (bf16 vals): 512B / 256B
REC1, REC2 = 136, 42        # [h|asrc|adst] cols used
SBUD = 48                   # slot budget per superstep (KA+KB)*gn
CH = 16                     # node tiles per write chunk

LAST_EXEC_NS = [0, 0]
LAST_BRS = []


def _wrap16(lin):
    n = lin.shape[0]
    arr = np.zeros((16, n // 16), np.int16)
    arr[np.arange(n) % 16, np.arange(n) // 16] = lin.astype(np.int16)
    return np.tile(arr, (8, 1))


def _r2(v):
    return max(2, int((v + 1) // 2 * 2))


def host_prep(edge_idx):
    src = np.concatenate([edge_idx[0], np.arange(N, dtype=np.int64)])
    dst = np.concatenate([edge_idx[1], np.arange(N, dtype=np.int64)])
    deg = np.bincount(dst, minlength=N)
    order = np.argsort(-deg, kind="stable")
    so = np.argsort(dst, kind="stable")
    src_s = src[so]
    starts = np.zeros(N + 1, np.int64)
    np.cumsum(deg, out=starts[1:])

    # per-node A/B src lists, self-first within its half
    listsA, listsB = [None] * N, [None] * N
    for d in range(N):
        seg = src_s[starts[d]:starts[d + 1]]
        a = seg[seg < SPLIT]
        b = seg[seg >= SPLIT]
        if d < SPLIT:
            i = int(np.nonzero(a == d)[0][0])
            if i:
                a = np.concatenate([[d], a[:i], a[i + 1:]])
        else:
            i = int(np.nonzero(b == d)[0][0])
            if i:
                b = np.concatenate([[d], b[:i], b[i + 1:]])
        listsA[d] = a
        listsB[d] = b - SPLIT

    nA = np.array([len(listsA[d]) for d in range(N)])
    nB = np.array([len(listsB[d]) for d in range(N)])
    # global sort by (-deg, -nA), pad, then deal strided into 8 cores so
    # every core's group g spans the same (deg, nA) range -> tight shared
    # (KA, KB) maxes across cores
    gs = np.lexsort((-nA, -deg))
    pad_node = gs[-1]
    glob = np.concatenate([gs, np.full(NG * 128 * NC_ - N, pad_node,
                                       np.int64)])
    blocks = glob.reshape(NG, 128 * NC_)
    core_dsts = [np.concatenate([blocks[g][c::NC_] for g in range(NG)])
                 for c in range(NC_)]
    KAj = np.zeros(NG, np.int64)
    KBj = np.zeros(NG, np.int64)
    for c in range(NC_):
        KAj = np.maximum(KAj, nA[core_dsts[c]].reshape(NG, 128).max(1))
        KBj = np.maximum(KBj, nB[core_dsts[c]].reshape(NG, 128).max(1))
    KAj = np.maximum(1, KAj)
    KBj = np.maximum(1, KBj)

    # supersteps: consecutive groups, same (KA,KB), (KA+KB)*gn <= SBUD
    sss = []
    j = 0
    while j < NG:
        KA, KB = KAj[j], KBj[j]
        gc = 1
        while (j + gc < NG and KAj[j + gc] == KA and KBj[j + gc] == KB
               and (gc + 1) * (KA + KB) <= SBUD):
            gc += 1
        sss.append((j, gc, int(KA), int(KB)))
        j += gc

    idxA, idxB, padc, maskA = [], [], [], []
    for c in range(NC_):
        linA_all, linB_all = [], []
        pc = np.zeros((128, NG), np.float32)
        mA = np.zeros((128, NG), np.float32)
        for (g0, gn, KA, KB) in sss:
            linA = np.full(gn * KA * 128, AROWS, np.int64)
            linB = np.full(gn * KB * 128, BROWS, np.int64)
            for gi in range(gn):
                g = g0 + gi
                for p in range(128):
                    d = core_dsts[c][g * 128 + p]
                    la, lb = listsA[d], listsB[d]
                    pc[p, g] = (KA - len(la)) + (KB - len(lb))
                    mA[p, g] = 1.0 if d < SPLIT else 0.0
                    o = (gi * KA) * 128 + p
                    linA[o:o + len(la) * 128:128] = la
                    o = (gi * KB) * 128 + p
                    linB[o:o + len(lb) * 128:128] = lb
            linA_all.append(_wrap16(linA))
            linB_all.append(_wrap16(linB))
        idxA.append(np.concatenate(linA_all, axis=1))
        idxB.append(np.concatenate(linB_all, axis=1))
        padc.append(pc)
        maskA.append(mA)
    meta = dict(sss=sss, core_dsts=core_dsts)
    return idxA, idxB, padc, maskA, meta


def _node_phase(nc, nod, ps, xt_in, we_in, TA, TB, nhalves, ELEM, REC):
    """h = x @ Wext for all nodes; bf16 records into split tables."""
    we = [nod.tile([128, REC], BF, name=f"we{h}") for h in range(nhalves)]
    for h in range(nhalves):
        nc.sync.dma_start(we[h][:], we_in[h * 128:(h + 1) * 128, :])
    zrow = nod.tile([1, ELEM], BF, name="zrow")
    nc.vector.memset(zrow[:], 0.0)
    nc.sync.dma_start(TA[AROWS:AROWS + 1, :], zrow[:])
    nc.sync.dma_start(TB[BROWS:BROWS + 1, :], zrow[:])

    nch = (NTILE + CH - 1) // CH
    for j in range(nch):
        t0 = j * CH
        nt = min(CH, NTILE - t0)
        cw = nt * 128
        xc = [nod.tile([128, CH * 128], BF, tag=f"xc{h}", name=f"xc{h}")
              for h in range(nhalves)]
        for h in range(nhalves):
            nc.sync.dma_start(xc[h][:, :cw],
                              xt_in[h * 128:(h + 1) * 128,
                                    t0 * 128:t0 * 128 + cw])
        st = nod.tile([128, CH * ELEM], BF, tag="st")
        for k in range(nt):
            ph = ps.tile([128, REC], FP, tag="ph")
            for h in range(nhalves):
                nc.tensor.matmul(ph[:], lhsT=xc[h][:, k * 128:(k + 1) * 128],
                                 rhs=we[h][:], start=(h == 0),
                                 stop=(h == nhalves - 1))
            nc.vector.tensor_copy(out=st[:, k * ELEM:k * ELEM + REC], in_=ph[:])
        # write records; split at table boundary (tile SPLIT//128)
        bt = SPLIT // 128  # 195
        r0, r1 = t0, t0 + nt
        if r0 < bt:
            ka = min(r1, bt) - r0
            nc.sync.dma_start(
                TA[r0 * 128:(r0 + ka) * 128, :].rearrange(
                    "(k p) e -> p k e", p=128),
                st[:, 0:ka * ELEM].rearrange("p (k e) -> p k e", e=ELEM))
        if r1 > bt:
            kb = r1 - max(r0, bt)
            ks = max(r0, bt) - r0
            b0 = max(r0, bt) - bt
            nc.sync.dma_start(
                TB[b0 * 128:(b0 + kb) * 128, :].rearrange(
                    "(k p) e -> p k e", p=128),
                st[:, ks * ELEM:(ks + kb) * ELEM].rearrange(
                    "p (k e) -> p k e", e=ELEM))


def _edge_phase(nc, ed, sss, idx_tA, idx_tB, TA, TB, ELEM, body):
    offA = offB = 0
    q = 0
    for si, (g0, gn, KA, KB) in enumerate(sss):
        nIA, nIB = gn * KA * 128, gn * KB * 128
        GA = ed.tile([128, gn * KA * ELEM], BF, tag="gA")
        GB = ed.tile([128, gn * KB * ELEM], BF, tag="gB")
        nc.gpsimd.dma_gather(GA[:].rearrange("p (s e) -> p s e", e=ELEM),
                             TA[:], idx_tA[:, offA:offA + nIA // 16],
                             nIA, nIA, ELEM, single_packet=False,
                             queue_num=q % 4)
        nc.gpsimd.dma_gather(GB[:].rearrange("p (s e) -> p s e", e=ELEM),
                             TB[:], idx_tB[:, offB:offB + nIB // 16],
                             nIB, nIB, ELEM, single_packet=False,
                             queue_num=(q + 1) % 4)
        q += 2
        offA += nIA // 16
        offB += nIB // 16
        body(si, GA, GB, g0, gn, KA, KB)


def build_l1(shapeA, shapeB, sss):
    nc = bacc.Bacc("TRN2", target_bir_lowering=False, num_swdge_queues=4)
    xt_in = nc.dram_tensor("xt", [IN, NPAD], BF, kind="ExternalInput")
    we_in = nc.dram_tensor("w1e", [IN, REC1], BF, kind="ExternalInput")
    ia_in = nc.dram_tensor("idxa", list(shapeA), I16, kind="ExternalInput")
    ib_in = nc.dram_tensor("idxb", list(shapeB), I16, kind="ExternalInput")
    pc_in = nc.dram_tensor("padc", [128, NG], FP, kind="ExternalInput")
    ma_in = nc.dram_tensor("maska", [128, NG], FP, kind="ExternalInput")
    out1 = nc.dram_tensor("out1", [NG * 128, 128], FP, kind="ExternalOutput")
    TA = nc.dram_tensor("ta", [AROWS + 1, ELEM1], BF, kind="Internal")
    TB = nc.dram_tensor("tb", [BROWS + 1, ELEM1], BF, kind="Internal")
    AF = mybir.ActivationFunctionType
    E = ELEM1

    with tile.TileContext(nc) as tc:
        with tc.tile_pool(name="cst", bufs=1) as cst, \
             tc.tile_pool(name="nod", bufs=3) as nod, \
             tc.tile_pool(name="ps", bufs=4, space="PSUM") as ps, \
             tc.tile_pool(name="ed", bufs=2) as ed:
            idx_tA = cst.tile(list(shapeA), I16)
            idx_tB = cst.tile(list(shapeB), I16)
            nc.sync.dma_start(idx_tA[:], ia_in[:])
            nc.sync.dma_start(idx_tB[:], ib_in[:])
            pc_t = cst.tile([128, NG], FP)
            nc.sync.dma_start(pc_t[:], pc_in[:])
            mA_t = cst.tile([128, NG], FP)
            nc.sync.dma_start(mA_t[:], ma_in[:])
            mB_t = cst.tile([128, NG], FP)
            nc.vector.tensor_scalar(out=mB_t[:], in0=mA_t[:], scalar1=-1.0,
                                    scalar2=1.0, op0=mybir.AluOpType.mult,
                                    op1=mybir.AluOpType.add)

            _node_phase(nc, nod, ps, xt_in, we_in, TA, TB, 2, ELEM1, REC1)

            def body(si, GA, GB, g0, gn, KA, KB):
                pA_ = GA[:].ap[0][0]
                pB_ = GB[:].ap[0][0]
                KT = KA + KB
                # eall: per group g interleaved [KA A-slots | KB B-slots] x 4
                # heads, plus gn*4 tail holding ad (al_dst from self slot 0)
                eall = ed.tile([128, gn * KT * 4 + gn * 4], FP, tag="eall")
                pe_ = eall[:].ap[0][0]
                toff = gn * KT * 4
                tmp = ed.tile([128, gn * 4], FP, tag="adB")
                nc.vector.tensor_tensor(
                    out=bass.AP(eall[:].tensor, eall[:].offset + toff,
                                [[pe_, 128], [4, gn], [1, 4]]),
                    in0=bass.AP(GA[:].tensor, GA[:].offset + 132,
                                [[pA_, 128], [KA * E, gn], [1, 4]]),
                    in1=bass.AP(mA_t[:].tensor, mA_t[:].offset + g0,
                                [[mA_t[:].ap[0][0], 128], [1, gn], [0, 4]]),
                    op=mybir.AluOpType.mult)
                nc.vector.tensor_tensor(
                    out=tmp[:].rearrange("p (g h) -> p g h", g=gn),
                    in0=bass.AP(GB[:].tensor, GB[:].offset + 132,
                                [[pB_, 128], [KB * E, gn], [1, 4]]),
                    in1=bass.AP(mB_t[:].tensor, mB_t[:].offset + g0,
                                [[mB_t[:].ap[0][0], 128], [1, gn], [0, 4]]),
                    op=mybir.AluOpType.mult)
                nc.vector.tensor_tensor(
                    out=bass.AP(eall[:].tensor, eall[:].offset + toff,
                                [[pe_, 128], [4, gn], [1, 4]]),
                    in0=bass.AP(eall[:].tensor, eall[:].offset + toff,
                                [[pe_, 128], [4, gn], [1, 4]]),
                    in1=tmp[:].rearrange("p (g h) -> p g h", g=gn),
                    op=mybir.AluOpType.add)
                for G, K, pg_, koff in ((GA, KA, pA_, 0), (GB, KB, pB_, KA)):
                    nc.vector.tensor_tensor(
                        out=bass.AP(eall[:].tensor, eall[:].offset + koff * 4,
                                    [[pe_, 128], [KT * 4, gn], [4, K],
                                     [1, 4]]),
                        in0=bass.AP(G[:].tensor, G[:].offset + 128,
                                    [[pg_, 128], [K * E, gn], [E, K], [1, 4]]),
                        in1=bass.AP(eall[:].tensor, eall[:].offset + toff,
                                    [[pe_, 128], [4, gn], [0, K], [1, 4]]),
                        op=mybir.AluOpType.add)
                nc.scalar.activation(eall[:], eall[:], AF.Lrelu, alpha=0.2)
                pall = ed.tile([128, gn * KT * 4 + gn * 4], BF, tag="pall")
                pp_ = pall[:].ap[0][0]
                nc.scalar.activation(pall[:], eall[:], AF.Exp)
                ssum = ed.tile([128, gn * 4], FP, tag="ssum")
                nc.vector.tensor_reduce(
                    out=ssum[:],
                    in_=bass.AP(pall[:].tensor, pall[:].offset,
                                [[pp_, 128], [KT * 4, gn], [1, 4], [4, KT]]),
                    axis=mybir.AxisListType.X, op=mybir.AluOpType.add)
                t1 = ed.tile([128, gn * 4], FP, tag="t1")
                nc.vector.tensor_tensor(
                    out=t1[:].rearrange("p (g h) -> p g h", g=gn),
                    in0=bass.AP(pall[:].tensor, pall[:].offset + toff,
                                [[pp_, 128], [4, gn], [1, 4]]),
                    in1=bass.AP(pc_t[:].tensor, pc_t[:].offset + g0,
                                [[pc_t[:].ap[0][0], 128], [1, gn], [0, 4]]),
                    op=mybir.AluOpType.mult)
                nc.vector.tensor_tensor(out=ssum[:], in0=ssum[:], in1=t1[:],
                                        op=mybir.AluOpType.subtract)
                rinv = ed.tile([128, gn * 4], FP, tag="rinv")
                nc.vector.reciprocal(rinv[:], ssum[:])
                gp = ed.tile([128, gn * KT * 128], BF, tag="gp", bufs=1)
                gp_ = gp[:].ap[0][0]
                for G, K, pg_, koff in ((GA, KA, pA_, 0), (GB, KB, pB_, KA)):
                    nc.vector.tensor_tensor(
                        out=bass.AP(gp[:].tensor, gp[:].offset + koff * 128,
                                    [[gp_, 128], [KT * 128, gn], [128, K],
                                     [32, 4], [1, 32]]),
                        in0=bass.AP(G[:].tensor, G[:].offset,
                                    [[pg_, 128], [K * E, gn], [E, K],
                                     [32, 4], [1, 32]]),
                        in1=bass.AP(pall[:].tensor, pall[:].offset + koff * 4,
                                    [[pp_, 128], [KT * 4, gn], [4, K],
                                     [1, 4], [0, 32]]),
                        op=mybir.AluOpType.mult)
                agg = ed.tile([128, gn * 128], FP, tag="agg", bufs=1)
                nc.vector.tensor_reduce(
                    out=agg[:],
                    in_=bass.AP(gp[:].tensor, gp[:].offset,
                                [[gp_, 128], [KT * 128, gn], [1, 128],
                                 [128, KT]]),
                    axis=mybir.AxisListType.X, op=mybir.AluOpType.add)
                outn = ed.tile([128, gn * 128], FP, tag="outn", bufs=1)
                nc.vector.tensor_tensor(
                    out=outn[:].rearrange("p (g h f) -> p g h f", g=gn, h=4),
                    in0=agg[:].rearrange("p (g h f) -> p g h f", g=gn, h=4),
                    in1=bass.AP(rinv[:].tensor, rinv[:].offset,
                                [[rinv[:].ap[0][0], 128], [4, gn],
                                 [1, 4], [0, 32]]),
                    op=mybir.AluOpType.mult)
                m0 = ed.tile([128, gn * 128], FP, tag="m0", bufs=1)
                nc.vector.tensor_scalar(out=m0[:], in0=outn[:], scalar1=0.0,
                                        scalar2=None, op0=mybir.AluOpType.min)
                nc.scalar.activation(m0[:], m0[:], AF.Exp)
                t3 = ed.tile([128, gn * 128], FP, tag="t3", bufs=1)
                nc.vector.tensor_scalar(out=t3[:], in0=outn[:], scalar1=0.0,
                                        scalar2=-1.0, op0=mybir.AluOpType.max,
                                        op1=mybir.AluOpType.add)
                nc.vector.tensor_tensor(out=t3[:], in0=t3[:], in1=m0[:],
                                        op=mybir.AluOpType.add)
                nc.sync.dma_start(
                    out1[g0 * 128:(g0 + gn) * 128, :].rearrange(
                        "(g p) f -> p g f", p=128),
                    t3[:].rearrange("p (g f) -> p g f", g=gn))

            _edge_phase(nc, ed, sss, idx_tA, idx_tB, TA, TB, ELEM1, body)
    nc.finalize()
    return nc


def build_l2(shapeA, shapeB, sss):
    nc = bacc.Bacc("TRN2", target_bir_lowering=False, num_swdge_queues=4)
    xt_in = nc.dram_tensor("h1t", [128, NPAD], BF, kind="ExternalInput")
    we_in = nc.dram_tensor("w2e", [128, REC2], BF, kind="ExternalInput")
    ia_in = nc.dram_tensor("idxa", list(shapeA), I16, kind="ExternalInput")
    ib_in = nc.dram_tensor("idxb", list(shapeB), I16, kind="ExternalInput")
    pc_in = nc.dram_tensor("padc", [128, NG], FP, kind="ExternalInput")
    ma_in = nc.dram_tensor("maska", [128, NG], FP, kind="ExternalInput")
    lg = nc.dram_tensor("logits", [NG * 128, OUT], FP, kind="ExternalOutput")
    TA = nc.dram_tensor("ta", [AROWS + 1, ELEM2], BF, kind="Internal")
    TB = nc.dram_tensor("tb", [BROWS + 1, ELEM2], BF, kind="Internal")
    AF = mybir.ActivationFunctionType
    E = ELEM2

    with tile.TileContext(nc) as tc:
        with tc.tile_pool(name="cst", bufs=1) as cst, \
             tc.tile_pool(name="nod", bufs=3) as nod, \
             tc.tile_pool(name="ps", bufs=4, space="PSUM") as ps, \
             tc.tile_pool(name="ed", bufs=2) as ed:
            idx_tA = cst.tile(list(shapeA), I16)
            idx_tB = cst.tile(list(shapeB), I16)
            nc.sync.dma_start(idx_tA[:], ia_in[:])
            nc.sync.dma_start(idx_tB[:], ib_in[:])
            pc_t = cst.tile([128, NG], FP)
            nc.sync.dma_start(pc_t[:], pc_in[:])
            mA_t = cst.tile([128, NG], FP)
            nc.sync.dma_start(mA_t[:], ma_in[:])
            mB_t = cst.tile([128, NG], FP)
            nc.vector.tensor_scalar(out=mB_t[:], in0=mA_t[:], scalar1=-1.0,
                                    scalar2=1.0, op0=mybir.AluOpType.mult,
                                    op1=mybir.AluOpType.add)

            _node_phase(nc, nod, ps, xt_in, we_in, TA, TB, 1, ELEM2, REC2)

            def body(si, GA, GB, g0, gn, KA, KB):
                pA_ = GA[:].ap[0][0]
                pB_ = GB[:].ap[0][0]
                KT = KA + KB
                eall = ed.tile([128, gn * KT + gn], FP, tag="eall")
                pe_ = eall[:].ap[0][0]
                toff = gn * KT
                tmp = ed.tile([128, gn], FP, tag="adB")
                nc.vector.tensor_tensor(
                    out=bass.AP(eall[:].tensor, eall[:].offset + toff,
                                [[pe_, 128], [1, gn]]),
                    in0=bass.AP(GA[:].tensor, GA[:].offset + 41,
                                [[pA_, 128], [KA * E, gn]]),
                    in1=mA_t[:, g0:g0 + gn], op=mybir.AluOpType.mult)
                nc.vector.tensor_tensor(
                    out=tmp[:],
                    in0=bass.AP(GB[:].tensor, GB[:].offset + 41,
                                [[pB_, 128], [KB * E, gn]]),
                    in1=mB_t[:, g0:g0 + gn], op=mybir.AluOpType.mult)
                nc.vector.tensor_tensor(
                    out=bass.AP(eall[:].tensor, eall[:].offset + toff,
                                [[pe_, 128], [1, gn]]),
                    in0=bass.AP(eall[:].tensor, eall[:].offset + toff,
                                [[pe_, 128], [1, gn]]),
                    in1=tmp[:], op=mybir.AluOpType.add)
                for G, K, pg_, koff in ((GA, KA, pA_, 0), (GB, KB, pB_, KA)):
                    nc.vector.tensor_tensor(
                        out=bass.AP(eall[:].tensor, eall[:].offset + koff,
                                    [[pe_, 128], [KT, gn], [1, K]]),
                        in0=bass.AP(G[:].tensor, G[:].offset + 40,
                                    [[pg_, 128], [K * E, gn], [E, K]]),
                        in1=bass.AP(eall[:].tensor, eall[:].offset + toff,
                                    [[pe_, 128], [1, gn], [0, K]]),
                        op=mybir.AluOpType.add)
                nc.scalar.activation(eall[:], eall[:], AF.Lrelu, alpha=0.2)
                pall = ed.tile([128, gn * KT + gn], BF, tag="pall")
                pp_ = pall[:].ap[0][0]
                nc.scalar.activation(pall[:], eall[:], AF.Exp)
                ssum = ed.tile([128, gn], FP, tag="ssum")
                nc.vector.tensor_reduce(
                    out=ssum[:],
                    in_=bass.AP(pall[:].tensor, pall[:].offset,
                                [[pp_, 128], [KT, gn], [1, KT]]),
                    axis=mybir.AxisListType.X, op=mybir.AluOpType.add)
                t1 = ed.tile([128, gn], FP, tag="t1")
                nc.vector.tensor_tensor(
                    out=t1[:],
                    in0=bass.AP(pall[:].tensor, pall[:].offset + toff,
                                [[pp_, 128], [1, gn]]),
                    in1=pc_t[:, g0:g0 + gn], op=mybir.AluOpType.mult)
                nc.vector.tensor_tensor(out=ssum[:], in0=ssum[:], in1=t1[:],
                                        op=mybir.AluOpType.subtract)
                rinv = ed.tile([128, gn], FP, tag="rinv")
                nc.vector.reciprocal(rinv[:], ssum[:])
                gp = ed.tile([128, gn * KT * OUT], BF, tag="gp", bufs=1)
                gp_ = gp[:].ap[0][0]
                for G, K, pg_, koff in ((GA, KA, pA_, 0), (GB, KB, pB_, KA)):
                    nc.vector.tensor_tensor(
                        out=bass.AP(gp[:].tensor, gp[:].offset + koff * OUT,
                                    [[gp_, 128], [KT * OUT, gn], [OUT, K],
                                     [1, OUT]]),
                        in0=bass.AP(G[:].tensor, G[:].offset,
                                    [[pg_, 128], [K * E, gn], [E, K],
                                     [1, OUT]]),
                        in1=bass.AP(pall[:].tensor, pall[:].offset + koff,
                                    [[pp_, 128], [KT, gn], [1, K],
                                     [0, OUT]]),
                        op=mybir.AluOpType.mult)
                agg = ed.tile([128, gn * OUT], FP, tag="agg", bufs=1)
                nc.vector.tensor_reduce(
                    out=agg[:],
                    in_=bass.AP(gp[:].tensor, gp[:].offset,
                                [[gp_, 128], [KT * OUT, gn], [1, OUT],
                                 [OUT, KT]]),
                    axis=mybir.AxisListType.X, op=mybir.AluOpType.add)
                out2 = ed.tile([128, gn * OUT], FP, tag="out2")
                nc.vector.tensor_tensor(
                    out=out2[:].rearrange("p (g f) -> p g f", g=gn),
                    in0=agg[:].rearrange("p (g f) -> p g f", g=gn),
                    in1=bass.AP(rinv[:].tensor, rinv[:].offset,
                                [[rinv[:].ap[0][0], 128], [1, gn], [0, OUT]]),
                    op=mybir.AluOpType.mult)
                ex = ed.tile([128, gn * OUT], FP, tag="ex", bufs=1)
                nc.scalar.activation(ex[:], out2[:], AF.Exp)
                se = ed.tile([128, gn], FP, tag="se")
                nc.vector.tensor_reduce(
                    out=se[:], in_=ex[:].rearrange("p (g f) -> p g f", g=gn),
                    axis=mybir.AxisListType.X, op=mybir.AluOpType.add)
                nc.scalar.activation(se[:], se[:], AF.Ln)
                nc.vector.tensor_tensor(
                    out=out2[:].rearrange("p (g f) -> p g f", g=gn),
                    in0=out2[:].rearrange("p (g f) -> p g f", g=gn),
                    in1=bass.AP(se[:].tensor, se[:].offset,
                                [[se[:].ap[0][0], 128], [1, gn], [0, OUT]]),
                    op=mybir.AluOpType.subtract)
                nc.sync.dma_start(
                    lg[g0 * 128:(g0 + gn) * 128, :].rearrange(
                        "(g p) f -> p g f", p=128),
                    out2[:].rearrange("p (g f) -> p g f", g=gn))

            _edge_phase(nc, ed, sss, idx_tA, idx_tB, TA, TB, ELEM2, body)
    nc.finalize()
    return nc


def kernel(x, edge_idx, W1, a_src1, a_dst1, b1, W2, a_src2, a_dst2, b2):
    x = np.asarray(x, np.float32)
    edge_idx = np.asarray(edge_idx)
    idxA, idxB, padc, maskA, meta = host_prep(edge_idx.astype(np.int64))
    sss, core_dsts = meta["sss"], meta["core_dsts"]

    abd = np.zeros((128, 8), np.float32)
    for h in range(H1):
        abd[h * C1:(h + 1) * C1, h] = np.asarray(a_src1, np.float32)[h]
        abd[h * C1:(h + 1) * C1, 4 + h] = np.asarray(a_dst1, np.float32)[h]
    W1f = np.asarray(W1, np.float32)
    w1e = np.concatenate([W1f, W1f @ abd], axis=1).astype(BF16)  # [256,136]
    a2bd = np.stack([np.asarray(a_src2, np.float32)[0],
                     np.asarray(a_dst2, np.float32)[0]], axis=1)  # [40,2]
    W2f = np.asarray(W2, np.float32)
    w2e = np.concatenate([W2f, W2f @ a2bd], axis=1).astype(BF16)  # [128,42]

    xt = np.zeros((IN, NPAD), BF16)
    xt[:, :N] = x.T.astype(BF16)

    shapeA, shapeB = idxA[0].shape, idxB[0].shape
    nc1 = build_l1(shapeA, shapeB, sss)
    in_maps = [{"xt": xt, "w1e": w1e, "idxa": idxA[c], "idxb": idxB[c],
                "padc": padc[c], "maska": maskA[c]} for c in range(NC_)]
    br1 = run_bass_kernel_spmd(nc1, in_maps, core_ids=list(range(NC_)),
                               trace=True)
    LAST_EXEC_NS[0] = br1.exec_time_ns or 0
    LAST_BRS.clear()
    LAST_BRS.append(br1)

    h1 = np.zeros((N, 128), np.float32)
    for c in range(NC_):
        h1[core_dsts[c][:NPC]] = br1.results[c]["out1"][:NPC]
    h1t = np.zeros((128, NPAD), BF16)
    h1t[:, :N] = h1.T.astype(BF16)

    nc2 = build_l2(shapeA, shapeB, sss)
    in_maps2 = [{"h1t": h1t, "w2e": w2e, "idxa": idxA[c], "idxb": idxB[c],
                 "padc": padc[c], "maska": maskA[c]} for c in range(NC_)]
    br2 = run_bass_kernel_spmd(nc2, in_maps2, core_ids=list(range(NC_)),
                               trace=True)
    LAST_EXEC_NS[1] = br2.exec_time_ns or 0
    LAST_BRS.append(br2)

    out = np.zeros((N, OUT), np.float32)
    for c in range(NC_):
        out[core_dsts[c][:NPC]] = br2.results[c]["logits"][:NPC]
    return out


# revision 13
# speedup vs baseline: 1.1210x; 1.1210x over previous
import sys, types
sys.path.insert(0, "/opt/trn_rl_repo")
import numpy as np
import ml_dtypes

BF16 = ml_dtypes.bfloat16


def _install_ntff_shim():
    try:
        import antenv  # noqa
        from trn_agent_boot.trn_boot import _ntff_profile_via_ctypes
        hook = _ntff_profile_via_ctypes('/opt/axon/libaxon_pjrt.so')
        m = types.ModuleType("antenv.axon_hooks")
        m.get_axon_ntff_profile_hook = lambda: hook
        m.set_axon_ntff_profile_hook = lambda h: None
        sys.modules["antenv.axon_hooks"] = m
    except Exception:
        pass
_install_ntff_shim()

from concourse import bass, mybir, tile, bacc
from concourse.bass_utils import run_bass_kernel_spmd

FP = mybir.dt.float32
BF = mybir.dt.bfloat16
I16 = mybir.dt.int16

N, IN, H1, C1, OUT = 50000, 256, 4, 32, 40
NC_ = 8
NPC = N // NC_              # 6250 dsts per core
NG = 49                     # groups of 128 dsts per core
SPLIT = 24960               # table A = nodes [0, SPLIT), B = [SPLIT, N)
NTILE = 391                 # ceil(N/128)
NPAD = NTILE * 128          # 50048
AROWS = SPLIT               # A table real rows; dummy at AROWS
BROWS = NPAD - SPLIT        # 25088 B rows (incl 48 fake); dummy at BROWS
ELEM1, ELEM2 = 256, 128     # gather elem (bf16 vals): 512B / 256B
REC1, REC2 = 136, 42        # [h|asrc|adst] cols used
SBUD = 48                   # slot budget per superstep (KA+KB)*gn
CH = 16                     # node tiles per write chunk

LAST_EXEC_NS = [0, 0]
LAST_BRS = []


def _wrap16(lin):
    n = lin.shape[0]
    arr = np.zeros((16, n // 16), np.int16)
    arr[np.arange(n) % 16, np.arange(n) // 16] = lin.astype(np.int16)
    return np.tile(arr, (8, 1))


def _r2(v):
    return max(2, int((v + 1) // 2 * 2))


def host_prep(edge_idx):
    src = np.concatenate([edge_idx[0], np.arange(N, dtype=np.int64)])
    dst = np.concatenate([edge_idx[1], np.arange(N, dtype=np.int64)])
    deg = np.bincount(dst, minlength=N)
    order = np.argsort(-deg, kind="stable")
    so = np.argsort(dst, kind="stable")
    src_s = src[so]
    starts = np.zeros(N + 1, np.int64)
    np.cumsum(deg, out=starts[1:])

    # per-node A/B src lists, self-first within its half
    listsA, listsB = [None] * N, [None] * N
    for d in range(N):
        seg = src_s[starts[d]:starts[d + 1]]
        a = seg[seg < SPLIT]
        b = seg[seg >= SPLIT]
        if d < SPLIT:
            i = int(np.nonzero(a == d)[0][0])
            if i:
                a = np.concatenate([[d], a[:i], a[i + 1:]])
        else:
            i = int(np.nonzero(b == d)[0][0])
            if i:
                b = np.concatenate([[d], b[:i], b[i + 1:]])
        listsA[d] = a
        listsB[d] = b - SPLIT

    nA = np.array([len(listsA[d]) for d in range(N)])
    nB = np.array([len(listsB[d]) for d in range(N)])
    # global sort by (-deg, -nA), pad, then deal strided into 8 cores so
    # every core's group g spans the same (deg, nA) range -> tight shared
    # (KA, KB) maxes across cores
    gs = np.lexsort((-nA, -deg))
    pad_node = gs[-1]
    glob = np.concatenate([gs, np.full(NG * 128 * NC_ - N, pad_node,
                                       np.int64)])
    blocks = glob.reshape(NG, 128 * NC_)
    core_dsts = [np.concatenate([blocks[g][c::NC_] for g in range(NG)])
                 for c in range(NC_)]
    KAj = np.zeros(NG, np.int64)
    KBj = np.zeros(NG, np.int64)
    for c in range(NC_):
        KAj = np.maximum(KAj, nA[core_dsts[c]].reshape(NG, 128).max(1))
        KBj = np.maximum(KBj, nB[core_dsts[c]].reshape(NG, 128).max(1))
    KAj = np.maximum(1, KAj)
    KBj = np.maximum(1, KBj)

    # supersteps: consecutive groups, same (KA,KB), (KA+KB)*gn <= SBUD
    sss = []
    j = 0
    while j < NG:
        KA, KB = KAj[j], KBj[j]
        gc = 1
        while (j + gc < NG and KAj[j + gc] == KA and KBj[j + gc] == KB
               and (gc + 1) * (KA + KB) <= SBUD):
            gc += 1
        sss.append((j, gc, int(KA), int(KB)))
        j += gc

    idxA, idxB, padc, maskA = [], [], [], []
    for c in range(NC_):
        linA_all, linB_all = [], []
        pc = np.zeros((128, NG), np.float32)
        mA = np.zeros((128, NG), np.float32)
        for (g0, gn, KA, KB) in sss:
            linA = np.full(gn * KA * 128, AROWS, np.int64)
            linB = np.full(gn * KB * 128, BROWS, np.int64)
            for gi in range(gn):
                g = g0 + gi
                for p in range(128):
                    d = core_dsts[c][g * 128 + p]
                    la, lb = listsA[d], listsB[d]
                    pc[p, g] = (KA - len(la)) + (KB - len(lb))
                    mA[p, g] = 1.0 if d < SPLIT else 0.0
                    o = (gi * KA) * 128 + p
                    linA[o:o + len(la) * 128:128] = la
                    o = (gi * KB) * 128 + p
                    linB[o:o + len(lb) * 128:128] = lb
            linA_all.append(_wrap16(linA))
            linB_all.append(_wrap16(linB))
        idxA.append(np.concatenate(linA_all, axis=1))
        idxB.append(np.concatenate(linB_all, axis=1))
        padc.append(pc)
        maskA.append(mA)
    meta = dict(sss=sss, core_dsts=core_dsts)
    return idxA, idxB, padc, maskA, meta


def _node_phase(nc, nod, ps, xt_in, we_in, TA, TB, nhalves, ELEM, REC):
    """h = x @ Wext for all nodes; bf16 records into split tables."""
    we = [nod.tile([128, REC], BF, name=f"we{h}") for h in range(nhalves)]
    for h in range(nhalves):
        nc.sync.dma_start(we[h][:], we_in[h * 128:(h + 1) * 128, :])
    zrow = nod.tile([1, ELEM], BF, name="zrow")
    nc.vector.memset(zrow[:], 0.0)
    nc.sync.dma_start(TA[AROWS:AROWS + 1, :], zrow[:])
    nc.sync.dma_start(TB[BROWS:BROWS + 1, :], zrow[:])

    nch = (NTILE + CH - 1) // CH
    for j in range(nch):
        t0 = j * CH
        nt = min(CH, NTILE - t0)
        cw = nt * 128
        xc = [nod.tile([128, CH * 128], BF, tag=f"xc{h}", name=f"xc{h}")
              for h in range(nhalves)]
        for h in range(nhalves):
            nc.sync.dma_start(xc[h][:, :cw],
                              xt_in[h * 128:(h + 1) * 128,
                                    t0 * 128:t0 * 128 + cw])
        st = nod.tile([128, CH * ELEM], BF, tag="st")
        for k in range(nt):
            ph = ps.tile([128, REC], FP, tag="ph")
            for h in range(nhalves):
                nc.tensor.matmul(ph[:], lhsT=xc[h][:, k * 128:(k + 1) * 128],
                                 rhs=we[h][:], start=(h == 0),
                                 stop=(h == nhalves - 1))
            nc.vector.tensor_copy(out=st[:, k * ELEM:k * ELEM + REC], in_=ph[:])
        # write records; split at table boundary (tile SPLIT//128)
        bt = SPLIT // 128  # 195
        r0, r1 = t0, t0 + nt
        if r0 < bt:
            ka = min(r1, bt) - r0
            nc.sync.dma_start(
                TA[r0 * 128:(r0 + ka) * 128, 0:REC].rearrange(
                    "(k p) e -> p k e", p=128),
                st[:, 0:ka * ELEM].rearrange(
                    "p (k e) -> p k e", e=ELEM)[:, :, 0:REC])
        if r1 > bt:
            kb = r1 - max(r0, bt)
            ks = max(r0, bt) - r0
            b0 = max(r0, bt) - bt
            nc.sync.dma_start(
                TB[b0 * 128:(b0 + kb) * 128, 0:REC].rearrange(
                    "(k p) e -> p k e", p=128),
                st[:, ks * ELEM:(ks + kb) * ELEM].rearrange(
                    "p (k e) -> p k e", e=ELEM)[:, :, 0:REC])


def _edge_phase(nc, ed, sss, idx_tA, idx_tB, TA, TB, ELEM, body):
    offA = offB = 0
    q = 0
    for si, (g0, gn, KA, KB) in enumerate(sss):
        nIA, nIB = gn * KA * 128, gn * KB * 128
        GA = ed.tile([128, gn * KA * ELEM], BF, tag="gA")
        GB = ed.tile([128, gn * KB * ELEM], BF, tag="gB")
        nc.gpsimd.dma_gather(GA[:].rearrange("p (s e) -> p s e", e=ELEM),
                             TA[:], idx_tA[:, offA:offA + nIA // 16],
                             nIA, nIA, ELEM, single_packet=False,
                             queue_num=q % 4)
        nc.gpsimd.dma_gather(GB[:].rearrange("p (s e) -> p s e", e=ELEM),
                             TB[:], idx_tB[:, offB:offB + nIB // 16],
                             nIB, nIB, ELEM, single_packet=False,
                             queue_num=(q + 1) % 4)
        q += 2
        offA += nIA // 16
        offB += nIB // 16
        body(si, GA, GB, g0, gn, KA, KB)


def build_l1(shapeA, shapeB, sss):
    nc = bacc.Bacc("TRN2", target_bir_lowering=False, num_swdge_queues=4)
    xt_in = nc.dram_tensor("xt", [IN, NPAD], BF, kind="ExternalInput")
    we_in = nc.dram_tensor("w1e", [IN, REC1], BF, kind="ExternalInput")
    ia_in = nc.dram_tensor("idxa", list(shapeA), I16, kind="ExternalInput")
    ib_in = nc.dram_tensor("idxb", list(shapeB), I16, kind="ExternalInput")
    pc_in = nc.dram_tensor("padc", [128, NG], FP, kind="ExternalInput")
    ma_in = nc.dram_tensor("maska", [128, NG], FP, kind="ExternalInput")
    out1 = nc.dram_tensor("out1", [NG * 128, 128], FP, kind="ExternalOutput")
    TA = nc.dram_tensor("ta", [AROWS + 1, ELEM1], BF, kind="Internal")
    TB = nc.dram_tensor("tb", [BROWS + 1, ELEM1], BF, kind="Internal")
    AF = mybir.ActivationFunctionType
    E = ELEM1

    with tile.TileContext(nc) as tc:
        with tc.tile_pool(name="cst", bufs=1) as cst, \
             tc.tile_pool(name="nod", bufs=3) as nod, \
             tc.tile_pool(name="ps", bufs=4, space="PSUM") as ps, \
             tc.tile_pool(name="ed", bufs=2) as ed:
            idx_tA = cst.tile(list(shapeA), I16)
            idx_tB = cst.tile(list(shapeB), I16)
            nc.sync.dma_start(idx_tA[:], ia_in[:])
            nc.sync.dma_start(idx_tB[:], ib_in[:])
            pc_t = cst.tile([128, NG], FP)
            nc.sync.dma_start(pc_t[:], pc_in[:])
            mA_t = cst.tile([128, NG], FP)
            nc.sync.dma_start(mA_t[:], ma_in[:])
            mB_t = cst.tile([128, NG], FP)
            nc.vector.tensor_scalar(out=mB_t[:], in0=mA_t[:], scalar1=-1.0,
                                    scalar2=1.0, op0=mybir.AluOpType.mult,
                                    op1=mybir.AluOpType.add)

            _node_phase(nc, nod, ps, xt_in, we_in, TA, TB, 2, ELEM1, REC1)

            def body(si, GA, GB, g0, gn, KA, KB):
                pA_ = GA[:].ap[0][0]
                pB_ = GB[:].ap[0][0]
                KT = KA + KB
                # eall: per group g interleaved [KA A-slots | KB B-slots] x 4
                # heads, plus gn*4 tail holding ad (al_dst from self slot 0)
                eall = ed.tile([128, gn * KT * 4 + gn * 4], FP, tag="eall")
                pe_ = eall[:].ap[0][0]
                toff = gn * KT * 4
                tmp = ed.tile([128, gn * 4], FP, tag="adB")
                nc.vector.tensor_tensor(
                    out=bass.AP(eall[:].tensor, eall[:].offset + toff,
                                [[pe_, 128], [4, gn], [1, 4]]),
                    in0=bass.AP(GA[:].tensor, GA[:].offset + 132,
                                [[pA_, 128], [KA * E, gn], [1, 4]]),
                    in1=bass.AP(mA_t[:].tensor, mA_t[:].offset + g0,
                                [[mA_t[:].ap[0][0], 128], [1, gn], [0, 4]]),
                    op=mybir.AluOpType.mult)
                nc.vector.tensor_tensor(
                    out=tmp[:].rearrange("p (g h) -> p g h", g=gn),
                    in0=bass.AP(GB[:].tensor, GB[:].offset + 132,
                                [[pB_, 128], [KB * E, gn], [1, 4]]),
                    in1=bass.AP(mB_t[:].tensor, mB_t[:].offset + g0,
                                [[mB_t[:].ap[0][0], 128], [1, gn], [0, 4]]),
                    op=mybir.AluOpType.mult)
                nc.vector.tensor_tensor(
                    out=bass.AP(eall[:].tensor, eall[:].offset + toff,
                                [[pe_, 128], [4, gn], [1, 4]]),
                    in0=bass.AP(eall[:].tensor, eall[:].offset + toff,
                                [[pe_, 128], [4, gn], [1, 4]]),
                    in1=tmp[:].rearrange("p (g h) -> p g h", g=gn),
                    op=mybir.AluOpType.add)
                for G, K, pg_, koff in ((GA, KA, pA_, 0), (GB, KB, pB_, KA)):
                    nc.vector.tensor_tensor(
                        out=bass.AP(eall[:].tensor, eall[:].offset + koff * 4,
                                    [[pe_, 128], [KT * 4, gn], [4, K],
                                     [1, 4]]),
                        in0=bass.AP(G[:].tensor, G[:].offset + 128,
                                    [[pg_, 128], [K * E, gn], [E, K], [1, 4]]),
                        in1=bass.AP(eall[:].tensor, eall[:].offset + toff,
                                    [[pe_, 128], [4, gn], [0, K], [1, 4]]),
                        op=mybir.AluOpType.add)
                lr = ed.tile([128, gn * KT * 4 + gn * 4], FP, tag="lr")
                nc.vector.tensor_scalar(out=lr[:], in0=eall[:], scalar1=0.2,
                                        scalar2=None, op0=mybir.AluOpType.mult)
                nc.vector.tensor_tensor(out=eall[:], in0=eall[:], in1=lr[:],
                                        op=mybir.AluOpType.max)
                pall = ed.tile([128, gn * KT * 4 + gn * 4], BF, tag="pall")
                pp_ = pall[:].ap[0][0]
                nc.scalar.activation(pall[:], eall[:], AF.Exp)
                ssum = ed.tile([128, gn * 4], FP, tag="ssum")
                nc.vector.tensor_reduce(
                    out=ssum[:],
                    in_=bass.AP(pall[:].tensor, pall[:].offset,
                                [[pp_, 128], [KT * 4, gn], [1, 4], [4, KT]]),
                    axis=mybir.AxisListType.X, op=mybir.AluOpType.add)
                t1 = ed.tile([128, gn * 4], FP, tag="t1")
                nc.vector.tensor_tensor(
                    out=t1[:].rearrange("p (g h) -> p g h", g=gn),
                    in0=bass.AP(pall[:].tensor, pall[:].offset + toff,
                                [[pp_, 128], [4, gn], [1, 4]]),
                    in1=bass.AP(pc_t[:].tensor, pc_t[:].offset + g0,
                                [[pc_t[:].ap[0][0], 128], [1, gn], [0, 4]]),
                    op=mybir.AluOpType.mult)
                nc.vector.tensor_tensor(out=ssum[:], in0=ssum[:], in1=t1[:],
                                        op=mybir.AluOpType.subtract)
                rinv = ed.tile([128, gn * 4], FP, tag="rinv")
                nc.vector.reciprocal(rinv[:], ssum[:])
                gp = ed.tile([128, gn * KT * 128], BF, tag="gp", bufs=1)
                gp_ = gp[:].ap[0][0]
                for G, K, pg_, koff in ((GA, KA, pA_, 0), (GB, KB, pB_, KA)):
                    nc.vector.tensor_tensor(
                        out=bass.AP(gp[:].tensor, gp[:].offset + koff * 128,
                                    [[gp_, 128], [KT * 128, gn], [128, K],
                                     [32, 4], [1, 32]]),
                        in0=bass.AP(G[:].tensor, G[:].offset,
                                    [[pg_, 128], [K * E, gn], [E, K],
                                     [32, 4], [1, 32]]),
                        in1=bass.AP(pall[:].tensor, pall[:].offset + koff * 4,
                                    [[pp_, 128], [KT * 4, gn], [4, K],
                                     [1, 4], [0, 32]]),
                        op=mybir.AluOpType.mult)
                agg = ed.tile([128, gn * 128], FP, tag="agg", bufs=1)
                nc.vector.tensor_reduce(
                    out=agg[:],
                    in_=bass.AP(gp[:].tensor, gp[:].offset,
                                [[gp_, 128], [KT * 128, gn], [1, 128],
                                 [128, KT]]),
                    axis=mybir.AxisListType.X, op=mybir.AluOpType.add)
                nc.vector.tensor_tensor(
                    out=hout[:, g0 * 128:(g0 + gn) * 128].rearrange(
                        "p (g h f) -> p g h f", g=gn, h=4),
                    in0=agg[:].rearrange("p (g h f) -> p g h f", g=gn, h=4),
                    in1=bass.AP(rinv[:].tensor, rinv[:].offset,
                                [[rinv[:].ap[0][0], 128], [4, gn],
                                 [1, 4], [0, 32]]),
                    op=mybir.AluOpType.mult)

            hout = cst.tile([128, NG * 128], FP)
            _edge_phase(nc, ed, sss, idx_tA, idx_tB, TA, TB, ELEM1, body)
            # deferred ELU over all dsts + single output DMA
            m0 = cst.tile([128, NG * 128], FP)
            nc.vector.tensor_scalar(out=m0[:], in0=hout[:], scalar1=0.0,
                                    scalar2=None, op0=mybir.AluOpType.min)
            nc.scalar.activation(m0[:], m0[:], AF.Exp)
            nc.vector.tensor_scalar(out=hout[:], in0=hout[:], scalar1=0.0,
                                    scalar2=-1.0, op0=mybir.AluOpType.max,
                                    op1=mybir.AluOpType.add)
            nc.vector.tensor_tensor(out=hout[:], in0=hout[:], in1=m0[:],
                                    op=mybir.AluOpType.add)
            nc.sync.dma_start(
                out1[:].rearrange("(g p) f -> p g f", p=128),
                hout[:].rearrange("p (g f) -> p g f", g=NG))
    nc.finalize()
    return nc


def build_l2(shapeA, shapeB, sss):
    nc = bacc.Bacc("TRN2", target_bir_lowering=False, num_swdge_queues=4)
    xt_in = nc.dram_tensor("h1t", [128, NPAD], BF, kind="ExternalInput")
    we_in = nc.dram_tensor("w2e", [128, REC2], BF, kind="ExternalInput")
    ia_in = nc.dram_tensor("idxa", list(shapeA), I16, kind="ExternalInput")
    ib_in = nc.dram_tensor("idxb", list(shapeB), I16, kind="ExternalInput")
    pc_in = nc.dram_tensor("padc", [128, NG], FP, kind="ExternalInput")
    ma_in = nc.dram_tensor("maska", [128, NG], FP, kind="ExternalInput")
    lg = nc.dram_tensor("logits", [NG * 128, OUT], FP, kind="ExternalOutput")
    TA = nc.dram_tensor("ta", [AROWS + 1, ELEM2], BF, kind="Internal")
    TB = nc.dram_tensor("tb", [BROWS + 1, ELEM2], BF, kind="Internal")
    AF = mybir.ActivationFunctionType
    E = ELEM2

    with tile.TileContext(nc) as tc:
        with tc.tile_pool(name="cst", bufs=1) as cst, \
             tc.tile_pool(name="nod", bufs=3) as nod, \
             tc.tile_pool(name="ps", bufs=4, space="PSUM") as ps, \
             tc.tile_pool(name="ed", bufs=2) as ed:
            idx_tA = cst.tile(list(shapeA), I16)
            idx_tB = cst.tile(list(shapeB), I16)
            nc.sync.dma_start(idx_tA[:], ia_in[:])
            nc.sync.dma_start(idx_tB[:], ib_in[:])
            pc_t = cst.tile([128, NG], FP)
            nc.sync.dma_start(pc_t[:], pc_in[:])
            mA_t = cst.tile([128, NG], FP)
            nc.sync.dma_start(mA_t[:], ma_in[:])
            mB_t = cst.tile([128, NG], FP)
            nc.vector.tensor_scalar(out=mB_t[:], in0=mA_t[:], scalar1=-1.0,
                                    scalar2=1.0, op0=mybir.AluOpType.mult,
                                    op1=mybir.AluOpType.add)

            _node_phase(nc, nod, ps, xt_in, we_in, TA, TB, 1, ELEM2, REC2)

            def body(si, GA, GB, g0, gn, KA, KB):
                pA_ = GA[:].ap[0][0]
                pB_ = GB[:].ap[0][0]
                KT = KA + KB
                eall = ed.tile([128, gn * KT + gn], FP, tag="eall")
                pe_ = eall[:].ap[0][0]
                toff = gn * KT
                tmp = ed.tile([128, gn], FP, tag="adB")
                nc.vector.tensor_tensor(
                    out=bass.AP(eall[:].tensor, eall[:].offset + toff,
                                [[pe_, 128], [1, gn]]),
                    in0=bass.AP(GA[:].tensor, GA[:].offset + 41,
                                [[pA_, 128], [KA * E, gn]]),
                    in1=mA_t[:, g0:g0 + gn], op=mybir.AluOpType.mult)
                nc.vector.tensor_tensor(
                    out=tmp[:],
                    in0=bass.AP(GB[:].tensor, GB[:].offset + 41,
                                [[pB_, 128], [KB * E, gn]]),
                    in1=mB_t[:, g0:g0 + gn], op=mybir.AluOpType.mult)
                nc.vector.tensor_tensor(
                    out=bass.AP(eall[:].tensor, eall[:].offset + toff,
                                [[pe_, 128], [1, gn]]),
                    in0=bass.AP(eall[:].tensor, eall[:].offset + toff,
                                [[pe_, 128], [1, gn]]),
                    in1=tmp[:], op=mybir.AluOpType.add)
                for G, K, pg_, koff in ((GA, KA, pA_, 0), (GB, KB, pB_, KA)):
                    nc.vector.tensor_tensor(
                        out=bass.AP(eall[:].tensor, eall[:].offset + koff,
                                    [[pe_, 128], [KT, gn], [1, K]]),
                        in0=bass.AP(G[:].tensor, G[:].offset + 40,
                                    [[pg_, 128], [K * E, gn], [E, K]]),
                        in1=bass.AP(eall[:].tensor, eall[:].offset + toff,
                                    [[pe_, 128], [1, gn], [0, K]]),
                        op=mybir.AluOpType.add)
                lr = ed.tile([128, gn * KT + gn], FP, tag="lr")
                nc.vector.tensor_scalar(out=lr[:], in0=eall[:], scalar1=0.2,
                                        scalar2=None, op0=mybir.AluOpType.mult)
                nc.vector.tensor_tensor(out=eall[:], in0=eall[:], in1=lr[:],
                                        op=mybir.AluOpType.max)
                pall = ed.tile([128, gn * KT + gn], BF, tag="pall")
                pp_ = pall[:].ap[0][0]
                nc.scalar.activation(pall[:], eall[:], AF.Exp)
                ssum = ed.tile([128, gn], FP, tag="ssum")
                nc.vector.tensor_reduce(
                    out=ssum[:],
                    in_=bass.AP(pall[:].tensor, pall[:].offset,
                                [[pp_, 128], [KT, gn], [1, KT]]),
                    axis=mybir.AxisListType.X, op=mybir.AluOpType.add)
                t1 = ed.tile([128, gn], FP, tag="t1")
                nc.vector.tensor_tensor(
                    out=t1[:],
                    in0=bass.AP(pall[:].tensor, pall[:].offset + toff,
                                [[pp_, 128], [1, gn]]),
                    in1=pc_t[:, g0:g0 + gn], op=mybir.AluOpType.mult)
                nc.vector.tensor_tensor(out=ssum[:], in0=ssum[:], in1=t1[:],
                                        op=mybir.AluOpType.subtract)
                rinv = ed.tile([128, gn], FP, tag="rinv")
                nc.vector.reciprocal(rinv[:], ssum[:])
                gp = ed.tile([128, gn * KT * OUT], BF, tag="gp", bufs=1)
                gp_ = gp[:].ap[0][0]
                for G, K, pg_, koff in ((GA, KA, pA_, 0), (GB, KB, pB_, KA)):
                    nc.vector.tensor_tensor(
                        out=bass.AP(gp[:].tensor, gp[:].offset + koff * OUT,
                                    [[gp_, 128], [KT * OUT, gn], [OUT, K],
                                     [1, OUT]]),
                        in0=bass.AP(G[:].tensor, G[:].offset,
                                    [[pg_, 128], [K * E, gn], [E, K],
                                     [1, OUT]]),
                        in1=bass.AP(pall[:].tensor, pall[:].offset + koff,
                                    [[pp_, 128], [KT, gn], [1, K],
                                     [0, OUT]]),
                        op=mybir.AluOpType.mult)
                agg = ed.tile([128, gn * OUT], FP, tag="agg", bufs=1)
                nc.vector.tensor_reduce(
                    out=agg[:],
                    in_=bass.AP(gp[:].tensor, gp[:].offset,
                                [[gp_, 128], [KT * OUT, gn], [1, OUT],
                                 [OUT, KT]]),
                    axis=mybir.AxisListType.X, op=mybir.AluOpType.add)
                nc.vector.tensor_tensor(
                    out=hout[:, g0 * OUT:(g0 + gn) * OUT].rearrange(
                        "p (g f) -> p g f", g=gn),
                    in0=agg[:].rearrange("p (g f) -> p g f", g=gn),
                    in1=bass.AP(rinv[:].tensor, rinv[:].offset,
                                [[rinv[:].ap[0][0], 128], [1, gn], [0, OUT]]),
                    op=mybir.AluOpType.mult)

            hout = cst.tile([128, NG * OUT], FP)
            _edge_phase(nc, ed, sss, idx_tA, idx_tB, TA, TB, ELEM2, body)
            # deferred log-softmax over all dsts + single output DMA
            ex = cst.tile([128, NG * OUT], FP)
            nc.scalar.activation(ex[:], hout[:], AF.Exp)
            se = cst.tile([128, NG], FP)
            nc.vector.tensor_reduce(
                out=se[:], in_=ex[:].rearrange("p (g f) -> p g f", g=NG),
                axis=mybir.AxisListType.X, op=mybir.AluOpType.add)
            nc.scalar.activation(se[:], se[:], AF.Ln)
            nc.vector.tensor_tensor(
                out=hout[:].rearrange("p (g f) -> p g f", g=NG),
                in0=hout[:].rearrange("p (g f) -> p g f", g=NG),
                in1=bass.AP(se[:].tensor, se[:].offset,
                            [[se[:].ap[0][0], 128], [1, NG], [0, OUT]]),
                op=mybir.AluOpType.subtract)
            nc.sync.dma_start(
                lg[:].rearrange("(g p) f -> p g f", p=128),
                hout[:].rearrange("p (g f) -> p g f", g=NG))
    nc.finalize()
    return nc


def kernel(x, edge_idx, W1, a_src1, a_dst1, b1, W2, a_src2, a_dst2, b2):
    x = np.asarray(x, np.float32)
    edge_idx = np.asarray(edge_idx)
    idxA, idxB, padc, maskA, meta = host_prep(edge_idx.astype(np.int64))
    sss, core_dsts = meta["sss"], meta["core_dsts"]

    abd = np.zeros((128, 8), np.float32)
    for h in range(H1):
        abd[h * C1:(h + 1) * C1, h] = np.asarray(a_src1, np.float32)[h]
        abd[h * C1:(h + 1) * C1, 4 + h] = np.asarray(a_dst1, np.float32)[h]
    W1f = np.asarray(W1, np.float32)
    w1e = np.concatenate([W1f, W1f @ abd], axis=1).astype(BF16)  # [256,136]
    a2bd = np.stack([np.asarray(a_src2, np.float32)[0],
                     np.asarray(a_dst2, np.float32)[0]], axis=1)  # [40,2]
    W2f = np.asarray(W2, np.float32)
    w2e = np.concatenate([W2f, W2f @ a2bd], axis=1).astype(BF16)  # [128,42]

    xt = np.zeros((IN, NPAD), BF16)
    xt[:, :N] = x.T.astype(BF16)

    shapeA, shapeB = idxA[0].shape, idxB[0].shape
    nc1 = build_l1(shapeA, shapeB, sss)
    in_maps = [{"xt": xt, "w1e": w1e, "idxa": idxA[c], "idxb": idxB[c],
                "padc": padc[c], "maska": maskA[c]} for c in range(NC_)]
    br1 = run_bass_kernel_spmd(nc1, in_maps, core_ids=list(range(NC_)),
                               trace=True)
    LAST_EXEC_NS[0] = br1.exec_time_ns or 0
    LAST_BRS.clear()
    LAST_BRS.append(br1)

    h1 = np.zeros((N, 128), np.float32)
    for c in range(NC_):
        h1[core_dsts[c][:NPC]] = br1.results[c]["out1"][:NPC]
    h1t = np.zeros((128, NPAD), BF16)
    h1t[:, :N] = h1.T.astype(BF16)

    nc2 = build_l2(shapeA, shapeB, sss)
    in_maps2 = [{"h1t": h1t, "w2e": w2e, "idxa": idxA[c], "idxb": idxB[c],
                 "padc": padc[c], "maska": maskA[c]} for c in range(NC_)]
    br2 = run_bass_kernel_spmd(nc2, in_maps2, core_ids=list(range(NC_)),
                               trace=True)
    LAST_EXEC_NS[1] = br2.exec_time_ns or 0
    LAST_BRS.append(br2)

    out = np.zeros((N, OUT), np.float32)
    for c in range(NC_):
        out[core_dsts[c][:NPC]] = br2.results[c]["logits"][:NPC]
    return out


# revision 15
# speedup vs baseline: 1.2933x; 1.1537x over previous
import sys, types
sys.path.insert(0, "/opt/trn_rl_repo")
import numpy as np
import ml_dtypes

BF16 = ml_dtypes.bfloat16


def _install_ntff_shim():
    try:
        import antenv  # noqa
        from trn_agent_boot.trn_boot import _ntff_profile_via_ctypes
        hook = _ntff_profile_via_ctypes('/opt/axon/libaxon_pjrt.so')
        m = types.ModuleType("antenv.axon_hooks")
        m.get_axon_ntff_profile_hook = lambda: hook
        m.set_axon_ntff_profile_hook = lambda h: None
        sys.modules["antenv.axon_hooks"] = m
    except Exception:
        pass
_install_ntff_shim()

from concourse import bass, mybir, tile, bacc
from concourse.bass_utils import run_bass_kernel_spmd

FP = mybir.dt.float32
BF = mybir.dt.bfloat16
I16 = mybir.dt.int16

N, IN, H1, C1, OUT = 50000, 256, 4, 32, 40
NC_ = 8
NPC = N // NC_              # 6250 dsts per core
NG = 49                     # groups of 128 dsts per core
SPLIT = 24960               # table A = nodes [0, SPLIT), B = [SPLIT, N)
NTILE = 391                 # ceil(N/128)
NPAD = NTILE * 128          # 50048
AROWS = SPLIT               # A table real rows; dummy at AROWS
BROWS = NPAD - SPLIT        # 25088 B rows (incl 48 fake); dummy at BROWS
ELEM1, ELEM2 = 256, 128     # gather elem (bf16 vals): 512B / 256B
REC1, REC2 = 136, 42        # [h|asrc|adst] cols used
SBUD = 48                   # slot budget per superstep (KA+KB)*gn
CH = 16                     # node tiles per write chunk

LAST_EXEC_NS = [0, 0]
LAST_BRS = []


def _wrap16(lin):
    n = lin.shape[0]
    arr = np.zeros((16, n // 16), np.int16)
    arr[np.arange(n) % 16, np.arange(n) // 16] = lin.astype(np.int16)
    return np.tile(arr, (8, 1))


def _r2(v):
    return max(2, int((v + 1) // 2 * 2))


def host_prep(edge_idx):
    src = np.concatenate([edge_idx[0], np.arange(N, dtype=np.int64)])
    dst = np.concatenate([edge_idx[1], np.arange(N, dtype=np.int64)])
    deg = np.bincount(dst, minlength=N)
    order = np.argsort(-deg, kind="stable")
    so = np.argsort(dst, kind="stable")
    src_s = src[so]
    starts = np.zeros(N + 1, np.int64)
    np.cumsum(deg, out=starts[1:])

    # per-node A/B src lists, self-first within its half
    listsA, listsB = [None] * N, [None] * N
    for d in range(N):
        seg = src_s[starts[d]:starts[d + 1]]
        a = seg[seg < SPLIT]
        b = seg[seg >= SPLIT]
        if d < SPLIT:
            i = int(np.nonzero(a == d)[0][0])
            if i:
                a = np.concatenate([[d], a[:i], a[i + 1:]])
        else:
            i = int(np.nonzero(b == d)[0][0])
            if i:
                b = np.concatenate([[d], b[:i], b[i + 1:]])
        listsA[d] = a
        listsB[d] = b - SPLIT

    nA = np.array([len(listsA[d]) for d in range(N)])
    nB = np.array([len(listsB[d]) for d in range(N)])
    # global sort by (-deg, -nA), pad, then deal strided into 8 cores so
    # every core's group g spans the same (deg, nA) range -> tight shared
    # (KA, KB) maxes across cores
    gs = np.lexsort((-nA, -deg))
    pad_node = gs[-1]
    glob = np.concatenate([gs, np.full(NG * 128 * NC_ - N, pad_node,
                                       np.int64)])
    blocks = glob.reshape(NG, 128 * NC_)
    core_dsts = [np.concatenate([blocks[g][c::NC_] for g in range(NG)])
                 for c in range(NC_)]
    KAj = np.zeros(NG, np.int64)
    KBj = np.zeros(NG, np.int64)
    for c in range(NC_):
        KAj = np.maximum(KAj, nA[core_dsts[c]].reshape(NG, 128).max(1))
        KBj = np.maximum(KBj, nB[core_dsts[c]].reshape(NG, 128).max(1))
    KAj = np.maximum(1, KAj)
    KBj = np.maximum(1, KBj)

    # supersteps: consecutive groups, same (KA,KB), (KA+KB)*gn <= SBUD
    sss = []
    j = 0
    while j < NG:
        KA, KB = KAj[j], KBj[j]
        gc = 1
        while (j + gc < NG and KAj[j + gc] == KA and KBj[j + gc] == KB
               and (gc + 1) * (KA + KB) <= SBUD):
            gc += 1
        sss.append((j, gc, int(KA), int(KB)))
        j += gc

    idxA, idxB, padc, maskA = [], [], [], []
    for c in range(NC_):
        linA_all, linB_all = [], []
        pc = np.zeros((128, NG), np.float32)
        mA = np.zeros((128, NG), np.float32)
        for (g0, gn, KA, KB) in sss:
            linA = np.full(gn * KA * 128, AROWS, np.int64)
            linB = np.full(gn * KB * 128, BROWS, np.int64)
            for gi in range(gn):
                g = g0 + gi
                for p in range(128):
                    d = core_dsts[c][g * 128 + p]
                    la, lb = listsA[d], listsB[d]
                    pc[p, g] = (KA - len(la)) + (KB - len(lb))
                    mA[p, g] = 1.0 if d < SPLIT else 0.0
                    o = (gi * KA) * 128 + p
                    linA[o:o + len(la) * 128:128] = la
                    o = (gi * KB) * 128 + p
                    linB[o:o + len(lb) * 128:128] = lb
            linA_all.append(_wrap16(linA))
            linB_all.append(_wrap16(linB))
        idxA.append(np.concatenate(linA_all, axis=1))
        idxB.append(np.concatenate(linB_all, axis=1))
        padc.append(pc)
        maskA.append(mA)
    meta = dict(sss=sss, core_dsts=core_dsts)
    return idxA, idxB, padc, maskA, meta


def _node_phase(nc, nod, ps, xt_in, we_in, TA, TB, nhalves, ELEM, REC):
    """h = x @ Wext for all nodes; bf16 records into split tables."""
    we = [nod.tile([128, REC], BF, name=f"we{h}") for h in range(nhalves)]
    for h in range(nhalves):
        nc.sync.dma_start(we[h][:], we_in[h * 128:(h + 1) * 128, :])
    zrow = nod.tile([1, ELEM], BF, name="zrow")
    nc.vector.memset(zrow[:], 0.0)
    nc.sync.dma_start(TA[AROWS:AROWS + 1, :], zrow[:])
    nc.sync.dma_start(TB[BROWS:BROWS + 1, :], zrow[:])

    nch = (NTILE + CH - 1) // CH
    for j in range(nch):
        t0 = j * CH
        nt = min(CH, NTILE - t0)
        cw = nt * 128
        xc = [nod.tile([128, CH * 128], BF, tag=f"xc{h}", name=f"xc{h}",
                       bufs=2) for h in range(nhalves)]
        for h in range(nhalves):
            nc.sync.dma_start(xc[h][:, :cw],
                              xt_in[h * 128:(h + 1) * 128,
                                    t0 * 128:t0 * 128 + cw])
        st = nod.tile([128, CH * REC], BF, tag="st", bufs=2)
        for k in range(nt):
            ph = ps.tile([128, REC], FP, tag="ph")
            for h in range(nhalves):
                nc.tensor.matmul(ph[:], lhsT=xc[h][:, k * 128:(k + 1) * 128],
                                 rhs=we[h][:], start=(h == 0),
                                 stop=(h == nhalves - 1))
            nc.vector.tensor_copy(out=st[:, k * REC:(k + 1) * REC], in_=ph[:])
        # write records; split at table boundary (tile SPLIT//128)
        bt = SPLIT // 128  # 195
        r0, r1 = t0, t0 + nt
        if r0 < bt:
            ka = min(r1, bt) - r0
            nc.sync.dma_start(
                TA[r0 * 128:(r0 + ka) * 128, 0:REC].rearrange(
                    "(k p) e -> p k e", p=128),
                st[:, 0:ka * REC].rearrange(
                    "p (k e) -> p k e", e=REC))
        if r1 > bt:
            kb = r1 - max(r0, bt)
            ks = max(r0, bt) - r0
            b0 = max(r0, bt) - bt
            nc.sync.dma_start(
                TB[b0 * 128:(b0 + kb) * 128, 0:REC].rearrange(
                    "(k p) e -> p k e", p=128),
                st[:, ks * REC:(ks + kb) * REC].rearrange(
                    "p (k e) -> p k e", e=REC))


def _edge_phase(nc, ed, sss, idx_tA, idx_tB, TA, TB, ELEM, body):
    offA = offB = 0
    q = 0
    for si, (g0, gn, KA, KB) in enumerate(sss):
        nIA, nIB = gn * KA * 128, gn * KB * 128
        GA = ed.tile([128, gn * KA * ELEM], BF, tag="gA", bufs=3)
        GB = ed.tile([128, gn * KB * ELEM], BF, tag="gB", bufs=3)
        nc.gpsimd.dma_gather(GA[:].rearrange("p (s e) -> p s e", e=ELEM),
                             TA[:], idx_tA[:, offA:offA + nIA // 16],
                             nIA, nIA, ELEM, single_packet=False,
                             queue_num=q % 4)
        nc.gpsimd.dma_gather(GB[:].rearrange("p (s e) -> p s e", e=ELEM),
                             TB[:], idx_tB[:, offB:offB + nIB // 16],
                             nIB, nIB, ELEM, single_packet=False,
                             queue_num=(q + 1) % 4)
        q += 2
        offA += nIA // 16
        offB += nIB // 16
        body(si, GA, GB, g0, gn, KA, KB)


def build_l1(shapeA, shapeB, sss):
    nc = bacc.Bacc("TRN2", target_bir_lowering=False, num_swdge_queues=4)
    xt_in = nc.dram_tensor("xt", [IN, NPAD], BF, kind="ExternalInput")
    we_in = nc.dram_tensor("w1e", [IN, REC1], BF, kind="ExternalInput")
    ia_in = nc.dram_tensor("idxa", list(shapeA), I16, kind="ExternalInput")
    ib_in = nc.dram_tensor("idxb", list(shapeB), I16, kind="ExternalInput")
    pc_in = nc.dram_tensor("padc", [128, NG], FP, kind="ExternalInput")
    ma_in = nc.dram_tensor("maska", [128, NG], FP, kind="ExternalInput")
    out1 = nc.dram_tensor("out1", [NG * 128, 128], FP, kind="ExternalOutput")
    TA = nc.dram_tensor("ta", [AROWS + 1, ELEM1], BF, kind="Internal")
    TB = nc.dram_tensor("tb", [BROWS + 1, ELEM1], BF, kind="Internal")
    AF = mybir.ActivationFunctionType
    E = ELEM1

    with tile.TileContext(nc) as tc:
        with tc.tile_pool(name="cst", bufs=1) as cst, \
             tc.tile_pool(name="nod", bufs=3) as nod, \
             tc.tile_pool(name="ps", bufs=4, space="PSUM") as ps, \
             tc.tile_pool(name="ed", bufs=2) as ed:
            idx_tA = cst.tile(list(shapeA), I16)
            idx_tB = cst.tile(list(shapeB), I16)
            nc.sync.dma_start(idx_tA[:], ia_in[:])
            nc.sync.dma_start(idx_tB[:], ib_in[:])
            pc_t = cst.tile([128, NG], FP)
            nc.sync.dma_start(pc_t[:], pc_in[:])
            mA_t = cst.tile([128, NG], FP)
            nc.sync.dma_start(mA_t[:], ma_in[:])
            mB_t = cst.tile([128, NG], FP)
            nc.vector.tensor_scalar(out=mB_t[:], in0=mA_t[:], scalar1=-1.0,
                                    scalar2=1.0, op0=mybir.AluOpType.mult,
                                    op1=mybir.AluOpType.add)

            _node_phase(nc, nod, ps, xt_in, we_in, TA, TB, 2, ELEM1, REC1)

            def body(si, GA, GB, g0, gn, KA, KB):
                pA_ = GA[:].ap[0][0]
                pB_ = GB[:].ap[0][0]
                KT = KA + KB
                # eall: per group g interleaved [KA A-slots | KB B-slots] x 4
                # heads, plus gn*4 tail holding ad (al_dst from self slot 0)
                eall = ed.tile([128, gn * KT * 4 + gn * 4], FP, tag="eall")
                pe_ = eall[:].ap[0][0]
                toff = gn * KT * 4
                tmp = ed.tile([128, gn * 4], FP, tag="adB")
                nc.vector.tensor_tensor(
                    out=bass.AP(eall[:].tensor, eall[:].offset + toff,
                                [[pe_, 128], [4, gn], [1, 4]]),
                    in0=bass.AP(GA[:].tensor, GA[:].offset + 132,
                                [[pA_, 128], [KA * E, gn], [1, 4]]),
                    in1=bass.AP(mA_t[:].tensor, mA_t[:].offset + g0,
                                [[mA_t[:].ap[0][0], 128], [1, gn], [0, 4]]),
                    op=mybir.AluOpType.mult)
                nc.vector.tensor_tensor(
                    out=tmp[:].rearrange("p (g h) -> p g h", g=gn),
                    in0=bass.AP(GB[:].tensor, GB[:].offset + 132,
                                [[pB_, 128], [KB * E, gn], [1, 4]]),
                    in1=bass.AP(mB_t[:].tensor, mB_t[:].offset + g0,
                                [[mB_t[:].ap[0][0], 128], [1, gn], [0, 4]]),
                    op=mybir.AluOpType.mult)
                nc.vector.tensor_tensor(
                    out=bass.AP(eall[:].tensor, eall[:].offset + toff,
                                [[pe_, 128], [4, gn], [1, 4]]),
                    in0=bass.AP(eall[:].tensor, eall[:].offset + toff,
                                [[pe_, 128], [4, gn], [1, 4]]),
                    in1=tmp[:].rearrange("p (g h) -> p g h", g=gn),
                    op=mybir.AluOpType.add)
                for G, K, pg_, koff in ((GA, KA, pA_, 0), (GB, KB, pB_, KA)):
                    nc.vector.tensor_tensor(
                        out=bass.AP(eall[:].tensor, eall[:].offset + koff * 4,
                                    [[pe_, 128], [KT * 4, gn], [4, K],
                                     [1, 4]]),
                        in0=bass.AP(G[:].tensor, G[:].offset + 128,
                                    [[pg_, 128], [K * E, gn], [E, K], [1, 4]]),
                        in1=bass.AP(eall[:].tensor, eall[:].offset + toff,
                                    [[pe_, 128], [4, gn], [0, K], [1, 4]]),
                        op=mybir.AluOpType.add)
                lr = ed.tile([128, gn * KT * 4 + gn * 4], FP, tag="lr")
                nc.vector.tensor_scalar(out=lr[:], in0=eall[:], scalar1=0.2,
                                        scalar2=None, op0=mybir.AluOpType.mult)
                nc.vector.tensor_tensor(out=eall[:], in0=eall[:], in1=lr[:],
                                        op=mybir.AluOpType.max)
                pall = ed.tile([128, gn * KT * 4 + gn * 4], BF, tag="pall")
                pp_ = pall[:].ap[0][0]
                nc.scalar.activation(pall[:], eall[:], AF.Exp)
                ssum = ed.tile([128, gn * 4], FP, tag="ssum")
                nc.vector.tensor_reduce(
                    out=ssum[:],
                    in_=bass.AP(pall[:].tensor, pall[:].offset,
                                [[pp_, 128], [KT * 4, gn], [1, 4], [4, KT]]),
                    axis=mybir.AxisListType.X, op=mybir.AluOpType.add)
                t1 = ed.tile([128, gn * 4], FP, tag="t1")
                nc.vector.tensor_tensor(
                    out=t1[:].rearrange("p (g h) -> p g h", g=gn),
                    in0=bass.AP(pall[:].tensor, pall[:].offset + toff,
                                [[pp_, 128], [4, gn], [1, 4]]),
                    in1=bass.AP(pc_t[:].tensor, pc_t[:].offset + g0,
                                [[pc_t[:].ap[0][0], 128], [1, gn], [0, 4]]),
                    op=mybir.AluOpType.mult)
                nc.vector.tensor_tensor(out=ssum[:], in0=ssum[:], in1=t1[:],
                                        op=mybir.AluOpType.subtract)
                rinv = ed.tile([128, gn * 4], FP, tag="rinv")
                nc.vector.reciprocal(rinv[:], ssum[:])
                gp = ed.tile([128, gn * KT * 128], BF, tag="gp", bufs=1)
                gp_ = gp[:].ap[0][0]
                for G, K, pg_, koff in ((GA, KA, pA_, 0), (GB, KB, pB_, KA)):
                    nc.vector.tensor_tensor(
                        out=bass.AP(gp[:].tensor, gp[:].offset + koff * 128,
                                    [[gp_, 128], [KT * 128, gn], [128, K],
                                     [32, 4], [1, 32]]),
                        in0=bass.AP(G[:].tensor, G[:].offset,
                                    [[pg_, 128], [K * E, gn], [E, K],
                                     [32, 4], [1, 32]]),
                        in1=bass.AP(pall[:].tensor, pall[:].offset + koff * 4,
                                    [[pp_, 128], [KT * 4, gn], [4, K],
                                     [1, 4], [0, 32]]),
                        op=mybir.AluOpType.mult)
                agg = ed.tile([128, gn * 128], FP, tag="agg", bufs=1)
                nc.vector.tensor_reduce(
                    out=agg[:],
                    in_=bass.AP(gp[:].tensor, gp[:].offset,
                                [[gp_, 128], [KT * 128, gn], [1, 128],
                                 [128, KT]]),
                    axis=mybir.AxisListType.X, op=mybir.AluOpType.add)
                nc.vector.tensor_tensor(
                    out=hout[:, g0 * 128:(g0 + gn) * 128].rearrange(
                        "p (g h f) -> p g h f", g=gn, h=4),
                    in0=agg[:].rearrange("p (g h f) -> p g h f", g=gn, h=4),
                    in1=bass.AP(rinv[:].tensor, rinv[:].offset,
                                [[rinv[:].ap[0][0], 128], [4, gn],
                                 [1, 4], [0, 32]]),
                    op=mybir.AluOpType.mult)

            hout = cst.tile([128, NG * 128], FP)
            _edge_phase(nc, ed, sss, idx_tA, idx_tB, TA, TB, ELEM1, body)
            # deferred ELU over all dsts + single output DMA
            m0 = cst.tile([128, NG * 128], BF)
            nc.vector.tensor_scalar(out=m0[:], in0=hout[:], scalar1=0.0,
                                    scalar2=None, op0=mybir.AluOpType.min)
            nc.scalar.activation(m0[:], m0[:], AF.Exp)
            nc.vector.tensor_scalar(out=hout[:], in0=hout[:], scalar1=0.0,
                                    scalar2=-1.0, op0=mybir.AluOpType.max,
                                    op1=mybir.AluOpType.add)
            nc.vector.tensor_tensor(out=hout[:], in0=hout[:], in1=m0[:],
                                    op=mybir.AluOpType.add)
            nc.sync.dma_start(
                out1[:].rearrange("(g p) f -> p g f", p=128),
                hout[:].rearrange("p (g f) -> p g f", g=NG))
    nc.finalize()
    return nc


def build_l2(shapeA, shapeB, sss):
    nc = bacc.Bacc("TRN2", target_bir_lowering=False, num_swdge_queues=4)
    xt_in = nc.dram_tensor("h1t", [128, NPAD], BF, kind="ExternalInput")
    we_in = nc.dram_tensor("w2e", [128, REC2], BF, kind="ExternalInput")
    ia_in = nc.dram_tensor("idxa", list(shapeA), I16, kind="ExternalInput")
    ib_in = nc.dram_tensor("idxb", list(shapeB), I16, kind="ExternalInput")
    pc_in = nc.dram_tensor("padc", [128, NG], FP, kind="ExternalInput")
    ma_in = nc.dram_tensor("maska", [128, NG], FP, kind="ExternalInput")
    lg = nc.dram_tensor("logits", [NG * 128, OUT], FP, kind="ExternalOutput")
    TA = nc.dram_tensor("ta", [AROWS + 1, ELEM2], BF, kind="Internal")
    TB = nc.dram_tensor("tb", [BROWS + 1, ELEM2], BF, kind="Internal")
    AF = mybir.ActivationFunctionType
    E = ELEM2

    with tile.TileContext(nc) as tc:
        with tc.tile_pool(name="cst", bufs=1) as cst, \
             tc.tile_pool(name="nod", bufs=3) as nod, \
             tc.tile_pool(name="ps", bufs=4, space="PSUM") as ps, \
             tc.tile_pool(name="ed", bufs=2) as ed:
            idx_tA = cst.tile(list(shapeA), I16)
            idx_tB = cst.tile(list(shapeB), I16)
            nc.sync.dma_start(idx_tA[:], ia_in[:])
            nc.sync.dma_start(idx_tB[:], ib_in[:])
            pc_t = cst.tile([128, NG], FP)
            nc.sync.dma_start(pc_t[:], pc_in[:])
            mA_t = cst.tile([128, NG], FP)
            nc.sync.dma_start(mA_t[:], ma_in[:])
            mB_t = cst.tile([128, NG], FP)
            nc.vector.tensor_scalar(out=mB_t[:], in0=mA_t[:], scalar1=-1.0,
                                    scalar2=1.0, op0=mybir.AluOpType.mult,
                                    op1=mybir.AluOpType.add)

            _node_phase(nc, nod, ps, xt_in, we_in, TA, TB, 1, ELEM2, REC2)

            def body(si, GA, GB, g0, gn, KA, KB):
                pA_ = GA[:].ap[0][0]
                pB_ = GB[:].ap[0][0]
                KT = KA + KB
                eall = ed.tile([128, gn * KT + gn], FP, tag="eall")
                pe_ = eall[:].ap[0][0]
                toff = gn * KT
                tmp = ed.tile([128, gn], FP, tag="adB")
                nc.vector.tensor_tensor(
                    out=bass.AP(eall[:].tensor, eall[:].offset + toff,
                                [[pe_, 128], [1, gn]]),
                    in0=bass.AP(GA[:].tensor, GA[:].offset + 41,
                                [[pA_, 128], [KA * E, gn]]),
                    in1=mA_t[:, g0:g0 + gn], op=mybir.AluOpType.mult)
                nc.vector.tensor_tensor(
                    out=tmp[:],
                    in0=bass.AP(GB[:].tensor, GB[:].offset + 41,
                                [[pB_, 128], [KB * E, gn]]),
                    in1=mB_t[:, g0:g0 + gn], op=mybir.AluOpType.mult)
                nc.vector.tensor_tensor(
                    out=bass.AP(eall[:].tensor, eall[:].offset + toff,
                                [[pe_, 128], [1, gn]]),
                    in0=bass.AP(eall[:].tensor, eall[:].offset + toff,
                                [[pe_, 128], [1, gn]]),
                    in1=tmp[:], op=mybir.AluOpType.add)
                for G, K, pg_, koff in ((GA, KA, pA_, 0), (GB, KB, pB_, KA)):
                    nc.vector.tensor_tensor(
                        out=bass.AP(eall[:].tensor, eall[:].offset + koff,
                                    [[pe_, 128], [KT, gn], [1, K]]),
                        in0=bass.AP(G[:].tensor, G[:].offset + 40,
                                    [[pg_, 128], [K * E, gn], [E, K]]),
                        in1=bass.AP(eall[:].tensor, eall[:].offset + toff,
                                    [[pe_, 128], [1, gn], [0, K]]),
                        op=mybir.AluOpType.add)
                lr = ed.tile([128, gn * KT + gn], FP, tag="lr")
                nc.vector.tensor_scalar(out=lr[:], in0=eall[:], scalar1=0.2,
                                        scalar2=None, op0=mybir.AluOpType.mult)
                nc.vector.tensor_tensor(out=eall[:], in0=eall[:], in1=lr[:],
                                        op=mybir.AluOpType.max)
                pall = ed.tile([128, gn * KT + gn], BF, tag="pall")
                pp_ = pall[:].ap[0][0]
                nc.scalar.activation(pall[:], eall[:], AF.Exp)
                ssum = ed.tile([128, gn], FP, tag="ssum")
                nc.vector.tensor_reduce(
                    out=ssum[:],
                    in_=bass.AP(pall[:].tensor, pall[:].offset,
                                [[pp_, 128], [KT, gn], [1, KT]]),
                    axis=mybir.AxisListType.X, op=mybir.AluOpType.add)
                t1 = ed.tile([128, gn], FP, tag="t1")
                nc.vector.tensor_tensor(
                    out=t1[:],
                    in0=bass.AP(pall[:].tensor, pall[:].offset + toff,
                                [[pp_, 128], [1, gn]]),
                    in1=pc_t[:, g0:g0 + gn], op=mybir.AluOpType.mult)
                nc.vector.tensor_tensor(out=ssum[:], in0=ssum[:], in1=t1[:],
                                        op=mybir.AluOpType.subtract)
                rinv = ed.tile([128, gn], FP, tag="rinv")
                nc.vector.reciprocal(rinv[:], ssum[:])
                gp = ed.tile([128, gn * KT * OUT], BF, tag="gp", bufs=1)
                gp_ = gp[:].ap[0][0]
                for G, K, pg_, koff in ((GA, KA, pA_, 0), (GB, KB, pB_, KA)):
                    nc.vector.tensor_tensor(
                        out=bass.AP(gp[:].tensor, gp[:].offset + koff * OUT,
                                    [[gp_, 128], [KT * OUT, gn], [OUT, K],
                                     [1, OUT]]),
                        in0=bass.AP(G[:].tensor, G[:].offset,
                                    [[pg_, 128], [K * E, gn], [E, K],
                                     [1, OUT]]),
                        in1=bass.AP(pall[:].tensor, pall[:].offset + koff,
                                    [[pp_, 128], [KT, gn], [1, K],
                                     [0, OUT]]),
                        op=mybir.AluOpType.mult)
                agg = ed.tile([128, gn * OUT], FP, tag="agg", bufs=1)
                nc.vector.tensor_reduce(
                    out=agg[:],
                    in_=bass.AP(gp[:].tensor, gp[:].offset,
                                [[gp_, 128], [KT * OUT, gn], [1, OUT],
                                 [OUT, KT]]),
                    axis=mybir.AxisListType.X, op=mybir.AluOpType.add)
                nc.vector.tensor_tensor(
                    out=hout[:, g0 * OUT:(g0 + gn) * OUT].rearrange(
                        "p (g f) -> p g f", g=gn),
                    in0=agg[:].rearrange("p (g f) -> p g f", g=gn),
                    in1=bass.AP(rinv[:].tensor, rinv[:].offset,
                                [[rinv[:].ap[0][0], 128], [1, gn], [0, OUT]]),
                    op=mybir.AluOpType.mult)

            hout = cst.tile([128, NG * OUT], FP)
            _edge_phase(nc, ed, sss, idx_tA, idx_tB, TA, TB, ELEM2, body)
            # deferred log-softmax over all dsts + single output DMA
            ex = cst.tile([128, NG * OUT], FP)
            nc.scalar.activation(ex[:], hout[:], AF.Exp)
            se = cst.tile([128, NG], FP)
            nc.vector.tensor_reduce(
                out=se[:], in_=ex[:].rearrange("p (g f) -> p g f", g=NG),
                axis=mybir.AxisListType.X, op=mybir.AluOpType.add)
            nc.scalar.activation(se[:], se[:], AF.Ln)
            nc.vector.tensor_tensor(
                out=hout[:].rearrange("p (g f) -> p g f", g=NG),
                in0=hout[:].rearrange("p (g f) -> p g f", g=NG),
                in1=bass.AP(se[:].tensor, se[:].offset,
                            [[se[:].ap[0][0], 128], [1, NG], [0, OUT]]),
                op=mybir.AluOpType.subtract)
            nc.sync.dma_start(
                lg[:].rearrange("(g p) f -> p g f", p=128),
                hout[:].rearrange("p (g f) -> p g f", g=NG))
    nc.finalize()
    return nc


def kernel(x, edge_idx, W1, a_src1, a_dst1, b1, W2, a_src2, a_dst2, b2):
    x = np.asarray(x, np.float32)
    edge_idx = np.asarray(edge_idx)
    idxA, idxB, padc, maskA, meta = host_prep(edge_idx.astype(np.int64))
    sss, core_dsts = meta["sss"], meta["core_dsts"]

    abd = np.zeros((128, 8), np.float32)
    for h in range(H1):
        abd[h * C1:(h + 1) * C1, h] = np.asarray(a_src1, np.float32)[h]
        abd[h * C1:(h + 1) * C1, 4 + h] = np.asarray(a_dst1, np.float32)[h]
    W1f = np.asarray(W1, np.float32)
    w1e = np.concatenate([W1f, W1f @ abd], axis=1).astype(BF16)  # [256,136]
    a2bd = np.stack([np.asarray(a_src2, np.float32)[0],
                     np.asarray(a_dst2, np.float32)[0]], axis=1)  # [40,2]
    W2f = np.asarray(W2, np.float32)
    w2e = np.concatenate([W2f, W2f @ a2bd], axis=1).astype(BF16)  # [128,42]

    xt = np.zeros((IN, NPAD), BF16)
    xt[:, :N] = x.T.astype(BF16)

    shapeA, shapeB = idxA[0].shape, idxB[0].shape
    nc1 = build_l1(shapeA, shapeB, sss)
    in_maps = [{"xt": xt, "w1e": w1e, "idxa": idxA[c], "idxb": idxB[c],
                "padc": padc[c], "maska": maskA[c]} for c in range(NC_)]
    br1 = run_bass_kernel_spmd(nc1, in_maps, core_ids=list(range(NC_)),
                               trace=True)
    LAST_EXEC_NS[0] = br1.exec_time_ns or 0
    LAST_BRS.clear()
    LAST_BRS.append(br1)

    h1 = np.zeros((N, 128), np.float32)
    for c in range(NC_):
        h1[core_dsts[c][:NPC]] = br1.results[c]["out1"][:NPC]
    h1t = np.zeros((128, NPAD), BF16)
    h1t[:, :N] = h1.T.astype(BF16)

    nc2 = build_l2(shapeA, shapeB, sss)
    in_maps2 = [{"h1t": h1t, "w2e": w2e, "idxa": idxA[c], "idxb": idxB[c],
                 "padc": padc[c], "maska": maskA[c]} for c in range(NC_)]
    br2 = run_bass_kernel_spmd(nc2, in_maps2, core_ids=list(range(NC_)),
                               trace=True)
    LAST_EXEC_NS[1] = br2.exec_time_ns or 0
    LAST_BRS.append(br2)

    out = np.zeros((N, OUT), np.float32)
    for c in range(NC_):
        out[core_dsts[c][:NPC]] = br2.results[c]["logits"][:NPC]
    return out
